# revision 38
# baseline (speedup 1.0000x reference)
"""Trainium2 Bass kernel for nn_Net_SLSTM_Conv (conv1d -> spiking LSTM -> BN ->
spiking LSTM -> mean -> fc), data-parallel over the T=512 axis on 8 cores.

Layout strategy (per core, T-chunk of 64 columns):
  - Everything feature-major: [features on partitions, t-columns on free dim].
  - Conv1d folded into one K=85 matmul (bf16 hi/lo split of x + ones row for
    bias); the xt3 DMA is chunked and conv matmuls are interleaved into the
    scan so step 0 starts as soon as chunk 0 lands.
  - Gate preactivations accumulate in a per-step PSUM bank [128, 4*64]
    (gates ordered g,i,f,o; gate g pre-scaled by 2 so one sigmoid op serves
    all four gates: tanh(x) = 2*sigmoid(2x)-1).
  - mem = o*tanh(syn) - thr*spk_prev is NEVER materialized: the recurrent
    matmul is split into Whh@mp (mp = o*tanh(syn), on the critical chain)
    plus (-thr*Whh)@spk_prev (off-chain, spikes known one step earlier).
    This drops one DVE op from the per-step dependency cycle.
  - Layer-1 spike counts accumulate per-step via a 64-column reduce placed
    in the DVE's idle window (no 17us end-of-scan reduce); BN normalization
    folds into layer-2 input weights/bias entirely on device (transpose
    matmul against a host identity; no DRAM round-trip).
  - fc bias enters the fc PSUM accumulation as a K=1 matmul against the
    xt3 ones row; the output DMAs from a plain DVE copy of the PSUM bank.
  - mean-over-steps + fc fold into an accumulating K=128->M=8 matmul pair
    (fcw@mp and -thr*fcw@spk).
"""
import os
import numpy as np
import ml_dtypes

import concourse.bass as bass
import concourse.mybir as mybir
import concourse.tile as tile
from concourse.bass_utils import run_bass_kernel_spmd

BF = mybir.dt.bfloat16
F32 = mybir.dt.float32
AF = mybir.ActivationFunctionType
OP = mybir.AluOpType

NCORES = 8
B, T, C = 256, 512, 14
H = 128
CH = 32          # conv output channels
TC = T // NCORES  # 64 t-columns per core
STEPS = int(os.environ.get("SLSTM_STEPS", B))  # debug override
EPS = 1e-5
GBUFS = 4        # PSUM step-bank rotation depth
NCONV = (B * TC) // 512       # conv chunks of 512 columns (= 8 steps each)
NDMA = 8                      # xt3 DMA chunks


def _bf16(x):
    return np.asarray(x, np.float32).astype(ml_dtypes.bfloat16)


def _reorder_gates_cols(wt):
    # [*, 512] gate-major cols in torch order i,f,g,o -> (g,i,f,o), scale g by 2
    i, f, g, o = (wt[..., k * H:(k + 1) * H] for k in range(4))
    return np.concatenate([2.0 * g, i, f, o], axis=-1)


def build_kernel(thr1: float, thr2: float):
    assert thr1 == 1.0 and thr2 == 1.0, "kernel specialized for thr == 1.0"
    nc = bass.Bass()

    # ---- external I/O ----
    xt3_d = nc.dram_tensor("xt3", [85, B * TC], BF, kind="ExternalInput")
    wconv_d = nc.dram_tensor("wconv", [85, CH], BF, kind="ExternalInput")
    w1t_d = nc.dram_tensor("w1t", [32, 4 * H], BF, kind="ExternalInput")
    w1b_d = nc.dram_tensor("w1b", [1, 4 * H], BF, kind="ExternalInput")
    whh1t_d = nc.dram_tensor("whh1t", [H, 4 * H], BF, kind="ExternalInput")
    w2t32_d = nc.dram_tensor("w2t32", [H, 4 * H], F32, kind="ExternalInput")
    w2tbf_d = nc.dram_tensor("w2tbf", [H, 4 * H], BF, kind="ExternalInput")
    whh2t_d = nc.dram_tensor("whh2t", [H, 4 * H], BF, kind="ExternalInput")
    b2sum4_d = nc.dram_tensor("b2sum4", [H, 4], F32, kind="ExternalInput")
    sel4_d = nc.dram_tensor("sel4", [4, 4 * TC], BF, kind="ExternalInput")
    ident_d = nc.dram_tensor("ident", [H, H], BF, kind="ExternalInput")
    fcwt_d = nc.dram_tensor("fcwt", [H, 8], BF, kind="ExternalInput")
    fcb_d = nc.dram_tensor("fcb", [1, 8], BF, kind="ExternalInput")
    gamma_d = nc.dram_tensor("gamma", [H, 1], F32, kind="ExternalInput")
    beta_d = nc.dram_tensor("beta", [H, 1], F32, kind="ExternalInput")
    out_d = nc.dram_tensor("out", [8, TC], F32, kind="ExternalOutput")
    DBG = bool(int(os.environ.get("SLSTM_DEBUG", "0")))
    if DBG:
        spk0_dd = nc.dram_tensor("spk0_d", [CH, B * TC], BF, kind="ExternalOutput")
        spk1_dd = nc.dram_tensor("spk1_d", [H, B * TC], BF, kind="ExternalOutput")
        cnt_dd = nc.dram_tensor("cnt_d", [H, 1], F32, kind="ExternalOutput")
        b2p_dd = nc.dram_tensor("b2p_d", [4, H], BF, kind="ExternalOutput")
        w2e_dd = nc.dram_tensor("w2e_d", [H, 4 * H], BF, kind="ExternalOutput")

    with tile.TileContext(nc) as tc:
        import contextlib
        ctx = contextlib.ExitStack()
        with ctx:
            const = ctx.enter_context(tc.tile_pool(name="const", bufs=1))
            big = ctx.enter_context(tc.tile_pool(name="big", bufs=1))
            spool = ctx.enter_context(tc.tile_pool(name="spool", bufs=3))
            vpool = ctx.enter_context(tc.tile_pool(name="vpool", bufs=3))
            stpool = ctx.enter_context(tc.tile_pool(name="stpool", bufs=3))
            gpool = ctx.enter_context(
                tc.tile_pool(name="gpool", bufs=GBUFS, space="PSUM"))
            cpool = ctx.enter_context(
                tc.tile_pool(name="cpool", bufs=2, space="PSUM"))
            fpool = ctx.enter_context(
                tc.tile_pool(name="fpool", bufs=1, space="PSUM"))
            dram = ctx.enter_context(
                tc.tile_pool(name="dram", bufs=1, space="DRAM"))

            # ---- load constants ----
            def load(pool, dt_, dram_t, shape):
                t_ = pool.tile(shape, dt_, name=dram_t.name + "_sb")
                nc.sync.dma_start(t_[:], dram_t[:])
                return t_

            # scan-critical loads first: conv weights + first xt3 chunk,
            # then layer-1 weights; everything else can trickle in behind.
            xt3_sb = big.tile([85, B * TC], BF, name="xt3_sb")
            DW = (B * TC) // NDMA
            def xt3_chunk(d):
                sl = slice(d * DW, (d + 1) * DW)
                nc.sync.dma_start(xt3_sb[:, sl], xt3_d[:, sl])
            wconv_sb = load(const, BF, wconv_d, [85, CH])
            xt3_chunk(0)
            w1t_sb = load(const, BF, w1t_d, [32, 4 * H])
            w1b_sb = load(const, BF, w1b_d, [1, 4 * H])
            whh1t_sb = load(const, BF, whh1t_d, [H, 4 * H])
            xt3_chunk(1)
            w2t32_sb = load(const, F32, w2t32_d, [H, 4 * H])
            w2tbf_sb = load(const, BF, w2tbf_d, [H, 4 * H])
            whh2t_sb = load(const, BF, whh2t_d, [H, 4 * H])
            b2sum4_sb = load(const, F32, b2sum4_d, [H, 4])
            xt3_chunk(2)
            sel4_sb = load(const, BF, sel4_d, [4, 4 * TC])
            ident_sb = load(const, BF, ident_d, [H, H])
            fcwt_sb = load(const, BF, fcwt_d, [H, 8])
            fcb_sb = load(const, BF, fcb_d, [1, 8])
            gamma_sb = load(const, F32, gamma_d, [H, 1])
            beta_sb = load(const, F32, beta_d, [H, 1])
            for d in range(3, NDMA):
                xt3_chunk(d)

            spk0_sb = big.tile([CH, B * TC], BF, name="spk0")
            spk1_sb = big.tile([H, B * TC], BF, name="spk1")
            zeros_sb = const.tile([H, TC], BF, name="zeros")
            nc.vector.memset(zeros_sb[:], 0.0)
            cntc_sb = const.tile([H, STEPS], F32, name="cntc")

            # ---- conv chunk: matmul + spike threshold ----
            def conv_chunk(cchunk):
                cp = cpool.tile([CH, 512], F32, name="convp", tag="convp")
                sl = slice(cchunk * 512, (cchunk + 1) * 512)
                nc.tensor.matmul(cp[:, :], wconv_sb[:, :], xt3_sb[:, sl],
                                 start=True, stop=True)
                nc.vector.tensor_scalar(spk0_sb[0:CH, sl], cp[:, :],
                                        1.0, 0.0, OP.subtract, OP.is_gt)

            conv_chunk(0)
            # derived constants, after the startup-critical conv work
            whh1s_sb = const.tile([H, 4 * H], BF, name="whh1s")
            nc.vector.tensor_scalar(whh1s_sb[:], whh1t_sb[:], -thr1, None, OP.mult)
            save = tc.cur_priority
            tc.cur_priority = save + 500000
            for c in range(1, min(4, NCONV)):
                conv_chunk(c)
            tc.cur_priority = save

            # ---- the recurrent step (shared between both layers) ----
            # state passed between steps: (syn, mp, spk, spk2) where
            #   mem_b = mp_b - thr*spk_{b-1}  (never materialized)
            def lstm_step(b, layer, syn_prev, mp_prev, spk_prev, spk_prev2, thr):
                gb = gpool.tile([H, 4 * TC], F32, name="gbank", tag="gbank")
                ones = xt3_sb[0:1, b * TC:(b + 1) * TC]
                started = False
                if layer == 2:
                    nc.tensor.matmul(gb[:, :], b2p_sb[:, :], sel4_sb[:, :],
                                     start=True, stop=False)
                    started = True
                    rhs_in = spk1_sb[:, b * TC:(b + 1) * TC]
                else:
                    rhs_in = spk0_sb[:, b * TC:(b + 1) * TC]
                for g in range(4):
                    lhs = (w2eff_sb[:, g * H:(g + 1) * H] if layer == 2
                           else w1t_sb[:, g * H:(g + 1) * H])
                    nc.tensor.matmul(gb[:, g * TC:(g + 1) * TC],
                                     lhs, rhs_in,
                                     start=not started and g == 0,
                                     stop=(layer == 2 and b == 0 and g == 3))
                if layer == 1:
                    for g in range(4):
                        nc.tensor.matmul(gb[:, g * TC:(g + 1) * TC],
                                         w1b_sb[:, g * H:(g + 1) * H],
                                         ones, start=False,
                                         stop=(b == 0 and g == 3))
                whh = whh1t_sb if layer == 1 else whh2t_sb
                whs = whh1s_sb if layer == 1 else whh2s_sb
                if b >= 2:
                    for g in range(4):
                        nc.tensor.matmul(gb[:, g * TC:(g + 1) * TC],
                                         whs[:, g * H:(g + 1) * H],
                                         spk_prev2[:, :],
                                         start=False, stop=False)
                if b >= 1:
                    for g in range(4):
                        nc.tensor.matmul(gb[:, g * TC:(g + 1) * TC],
                                         whh[:, g * H:(g + 1) * H],
                                         mp_prev[:, :],
                                         start=False, stop=(g == 3))
                # gate order in bank: g' | i | f | o
                S = spool.tile([H, 4 * TC], BF, name="S", tag="S")
                nc.scalar.activation(S[:, 0:3 * TC], gb[:, 0:3 * TC], AF.Sigmoid)
                nc.scalar.activation(S[:, 3 * TC:], gb[:, 3 * TC:], AF.Sigmoid)
                # critical DVE block. State h == syn/2 so the combiner is a
                # plain TT add (2x DVE mode) instead of a 1x STT, and the
                # missing factor 2 rides the tanh's input scale for free:
                #   h_b = f*h_{b-1} + u,  u = (sig(2g)-0.5)*i = i*g/2
                #   ts  = tanh(2*h) = tanh(syn)
                syn = stpool.tile([H, TC], BF, name="syn", tag="syn")
                u = vpool.tile([H, TC], BF, name="u", tag="u")
                if b >= 1:
                    fs = vpool.tile([H, TC], BF, name="fs", tag="fs")
                    nc.vector.tensor_tensor(fs[:], S[:, 2 * TC:3 * TC],
                                            syn_prev[:], op=OP.mult)
                    nc.vector.scalar_tensor_tensor(
                        u[:], S[:, 0:TC], 0.5, S[:, TC:2 * TC],
                        op0=OP.subtract, op1=OP.mult)
                    nc.vector.tensor_tensor(syn[:], u[:], fs[:], op=OP.add)
                else:
                    nc.vector.scalar_tensor_tensor(
                        u[:], S[:, 0:TC], 0.5, S[:, TC:2 * TC],
                        op0=OP.subtract, op1=OP.mult)
                    nc.vector.tensor_tensor(syn[:], u[:], zeros_sb[:],
                                            op=OP.add)
                ts = vpool.tile([H, TC], BF, name="ts", tag="ts")
                nc.scalar.activation(ts[:], syn[:], AF.Tanh, scale=2.0)
                mp = stpool.tile([H, TC], BF, name="mp", tag="mp")
                nc.vector.tensor_tensor(mp[:], S[:, 3 * TC:4 * TC], ts[:],
                                        op=OP.mult)
                # spike off the critical chain: spk = (mp - thr) > thr*spk_prev
                # (thr == 1.0 so stored spikes are already thr-scaled)
                if layer == 1:
                    spk = spk1_sb[:, b * TC:(b + 1) * TC]
                else:
                    spk = stpool.tile([H, TC], BF, name="spk2", tag="spk2")[:]
                if b >= 1:
                    nc.vector.scalar_tensor_tensor(
                        spk, mp[:], thr, spk_prev, op0=OP.subtract,
                        op1=OP.is_gt)
                else:
                    nc.vector.tensor_scalar(spk, mp[:], thr, 0.0,
                                            OP.subtract, OP.is_gt)
                if layer == 1:
                    # per-step spike count column (off the critical chain)
                    nc.vector.tensor_reduce(cntc_sb[:, b:b + 1], spk,
                                            axis=mybir.AxisListType.X,
                                            op=OP.add)
                return syn, mp, spk

            # ---- phase A: layer-1 scan ----
            syn_p, mp_p = zeros_sb, zeros_sb
            spk_p, spk_p2 = zeros_sb[:], zeros_sb[:]
            for b in range(STEPS):
                with tc.high_priority():
                    syn_p, mp_p, spk_n = lstm_step(b, 1, syn_p, mp_p, spk_p,
                                                   spk_p2, thr1)
                spk_p2, spk_p = spk_p, spk_n
                c = b // 8 + 4
                if b % 8 == 0 and c < NCONV:
                    # deprioritize conv threshold work vs the chain
                    save = tc.cur_priority
                    tc.cur_priority = save + 500000
                    conv_chunk(c)
                    tc.cur_priority = save

            # ---- BN stats: count -> AllReduce -> fold into layer-2 weights ----
            cnt = const.tile([H, 1], F32, name="cnt")
            nc.vector.tensor_reduce(cnt[:], cntc_sb[:, 0:STEPS],
                                    axis=mybir.AxisListType.X, op=OP.add)
            cc_in = dram.tile([H, 1], F32, name="cc_in")
            cc_out = dram.tile([H, 1], F32, name="cc_out", addr_space="Shared")
            nc.sync.dma_start(cc_in[:], cnt[:])
            nc.gpsimd.collective_compute(
                "AllReduce", OP.add,
                replica_groups=[list(range(NCORES))],
                ins=[cc_in[:]], outs=[cc_out[:]])
            cntg = const.tile([H, 1], F32, name="cntg")
            nc.sync.dma_start(cntg[:], cc_out[:])

            p_t = const.tile([H, 1], F32, name="p_t")
            nc.vector.tensor_scalar(p_t[:], cntg[:], 1.0 / (B * T), None, OP.mult)
            q_t = const.tile([H, 1], F32, name="q_t")
            nc.vector.tensor_scalar(q_t[:], p_t[:], -1.0, 1.0, OP.mult, OP.add)
            var_t = const.tile([H, 1], F32, name="var_t")
            nc.vector.tensor_tensor(var_t[:], p_t[:], q_t[:], op=OP.mult)
            nc.vector.tensor_scalar(var_t[:], var_t[:], EPS, None, OP.add)
            sq_t = const.tile([H, 1], F32, name="sq_t")
            nc.scalar.activation(sq_t[:], var_t[:], AF.Sqrt, bias=0.0)
            rs_t = const.tile([H, 1], F32, name="rs_t")
            nc.vector.reciprocal(rs_t[:], sq_t[:])
            a_t = const.tile([H, 1], F32, name="a_t")
            nc.vector.tensor_tensor(a_t[:], gamma_sb[:], rs_t[:], op=OP.mult)
            pa_t = const.tile([H, 1], F32, name="pa_t")
            nc.vector.tensor_tensor(pa_t[:], p_t[:], a_t[:], op=OP.mult)
            c_t = const.tile([H, 1], F32, name="c_t")
            nc.vector.scalar_tensor_tensor(c_t[:], pa_t[:], -1.0, beta_sb[:],
                                           op0=OP.mult, op1=OP.add)
            cbf_t = const.tile([H, 1], BF, name="cbf_t")
            nc.vector.tensor_copy(cbf_t[:], c_t[:])

            w2eff_sb = const.tile([H, 4 * H], BF, name="w2eff")
            nc.vector.tensor_scalar(w2eff_sb[:], w2t32_sb[:], a_t[:], None, OP.mult)
            whh2s_sb = const.tile([H, 4 * H], BF, name="whh2s")
            nc.vector.tensor_scalar(whh2s_sb[:], whh2t_sb[:], -thr2, None, OP.mult)
            fcwts_sb = const.tile([H, 8], BF, name="fcwts")
            nc.vector.tensor_scalar(fcwts_sb[:], fcwt_sb[:], -thr2, None, OP.mult)

            # layer-2 bias = W2 @ c + b2sum, built fully on device:
            # bias2[h, g] via 4 single-column matmuls, + b2sum4; then
            # transpose to [4, H] with a matmul against the identity.
            bias2p = cpool.tile([H, 4], F32, name="bias2p", tag="convp")
            for g in range(4):
                nc.tensor.matmul(bias2p[:, g:g + 1],
                                 w2tbf_sb[:, g * H:(g + 1) * H], cbf_t[:, :],
                                 start=True, stop=True)
            bias2bf = const.tile([H, 4], BF, name="bias2bf")
            nc.vector.tensor_tensor(bias2bf[:], bias2p[:, :], b2sum4_sb[:],
                                    op=OP.add)
            b2pp = cpool.tile([4, H], F32, name="b2pp", tag="convp")
            nc.tensor.matmul(b2pp[:, :], bias2bf[:, :], ident_sb[:, :],
                             start=True, stop=True)
            b2p_sb = const.tile([4, H], BF, name="b2p")
            nc.vector.tensor_copy(b2p_sb[:], b2pp[:, :])

            # ---- phase B: layer-2 scan, fused mean+fc accumulation ----
            fcp = fpool.tile([8, TC], F32, name="fcp", tag="fcp")
            nc.tensor.matmul(fcp[:, :], fcb_sb[:, :], xt3_sb[0:1, 0:TC],
                             start=True, stop=False)
            syn_p, mp_p = zeros_sb, zeros_sb
            spk_p, spk_p2 = zeros_sb[:], zeros_sb[:]

            # fc accumulation: sum_b mem_b = sum_b mp_b - thr*sum spk_{b-1};
            # emitted one step late so the PE queue isn't stalled on mp_b
            # before the next step's early (input/bias/spk) matmuls.
            def fc_mms(b, mp_b, spk_bm1):
                nc.tensor.matmul(fcp[:, :], fcwt_sb[:, :], mp_b[:, :],
                                 start=False, stop=False)
                if b >= 1:
                    nc.tensor.matmul(fcp[:, :], fcwts_sb[:, :], spk_bm1,
                                     start=False, stop=(b == STEPS - 1))

            for b in range(STEPS):
                prev = (b - 1, mp_p, spk_p2)  # (k, mp_k, spk_{k-1})
                syn_p, mp_p, spk_n = lstm_step(b, 2, syn_p, mp_p, spk_p,
                                               spk_p2, thr2)
                if b >= 1:
                    fc_mms(prev[0], prev[1], prev[2])
                spk_p2, spk_p = spk_p, spk_n
            fc_mms(STEPS - 1, mp_p, spk_p2)

            out_sb = const.tile([8, TC], F32, name="out_sb")
            nc.vector.tensor_copy(out_sb[:], fcp[:, :])
            nc.sync.dma_start(out_d[:], out_sb[:])

            if DBG:
                nc.sync.dma_start(spk0_dd[:], spk0_sb[:])
                nc.sync.dma_start(spk1_dd[:], spk1_sb[:])
                nc.sync.dma_start(cnt_dd[:], cnt[:])
                nc.sync.dma_start(b2p_dd[:], b2p_sb[:])
                nc.sync.dma_start(w2e_dd[:], w2eff_sb[:])

    _split_mm_waits(nc)
    return nc


def _split_mm_waits(nc):
    """Most ISA structs carry a single sync-wait slot, so extra Tile-assigned
    waits must move onto preceding same-engine NoOps. Choose the KEPT wait to
    be the one whose producing update appears LATEST in program order (the
    likely critical dependency); early-satisfied waits go to the NoOps, which
    then drain instantly instead of serializing the critical path."""
    for fn in nc.m.functions:
        for blk in fn.blocks:
            # pass 1: per-semaphore cumulative update positions
            sem_updates = {}  # (name,id) -> list of (cum_value, pos)
            for pos, inst in enumerate(blk.instructions):
                si = getattr(inst, "sync_info", None)
                if si is None:
                    continue
                for upd in (si.on_update or []):
                    key = (upd.ant_name, upd.id)
                    lst = sem_updates.setdefault(key, [])
                    prev = lst[-1][0] if lst else 0
                    val = getattr(upd, "value", 1) or 1
                    lst.append((prev + val, pos))

            def producer_pos(w):
                key = (w.ant_name, w.id)
                lst = sem_updates.get(key)
                tgt = getattr(w, "wait_value", None)
                if not lst or tgt is None:
                    return -1
                lo, hi = 0, len(lst)
                while lo < hi:
                    mid = (lo + hi) // 2
                    if lst[mid][0] >= tgt:
                        hi = mid
                    else:
                        lo = mid + 1
                return lst[lo][1] if lo < len(lst) else 10 ** 9

            out = []
            for inst in blk.instructions:
                si = getattr(inst, "sync_info", None)
                if (not isinstance(inst, (mybir.InstEventSemaphore,
                                          mybir.InstAllEngineBarrier,
                                          mybir.InstNoOp))
                        and si is not None and si.on_wait
                        and len(si.on_wait) > 1):
                    waits = sorted(si.on_wait, key=producer_pos)
                    for j, w in enumerate(waits[:-1]):
                        nop = mybir.InstNoOp(name=f"{inst.name}-wsplit{j}",
                                             ins=[], outs=[])
                        nop.engine = inst.engine
                        nop.sync_info = mybir.SyncInfo(on_wait=[w],
                                                       on_update=[])
                        out.append(nop)
                    si.on_wait = [waits[-1]]
                out.append(inst)
            blk.instructions[:] = out


def _host_inputs(x, conv_w, conv_b, w_ih1, w_hh1, b_ih1, b_hh1,
                 w_ih2, w_hh2, b_ih2, b_hh2, bn_gamma, bn_beta, fc_w, fc_b):
    """Build the per-core input dicts (numpy, host-side)."""
    f32 = np.float32
    # im2col with hi/lo bf16 split per core
    xp = np.pad(np.asarray(x, f32), ((0, 0), (1, 1), (0, 0)))  # [B, T+2, C]
    common = {}
    w3t = np.concatenate([conv_w[:, :, k].T for k in range(3)], axis=0)  # [42,32]
    common["wconv"] = _bf16(np.concatenate(
        [np.asarray(conv_b, f32)[None, :], w3t, w3t], axis=0))
    w1t = _reorder_gates_cols(np.asarray(w_ih1, f32).T)        # [32, 512]
    b1 = _reorder_gates_cols((np.asarray(b_ih1) + np.asarray(b_hh1))[None, :])
    common["w1t"] = _bf16(w1t)                                 # [32, 512]
    common["w1b"] = _bf16(b1)                                  # [1, 512]
    common["whh1t"] = _bf16(_reorder_gates_cols(np.asarray(w_hh1, f32).T))
    w2t = _reorder_gates_cols(np.asarray(w_ih2, f32).T)        # [128, 512]
    common["w2t32"] = np.ascontiguousarray(w2t, f32)
    common["w2tbf"] = _bf16(w2t)
    common["whh2t"] = _bf16(_reorder_gates_cols(np.asarray(w_hh2, f32).T))
    b2r = _reorder_gates_cols((np.asarray(b_ih2) + np.asarray(b_hh2))[None, :])
    common["b2sum4"] = np.ascontiguousarray(b2r.reshape(4, H).T, f32)  # [H, 4]
    sel = np.zeros((4, 4 * TC), f32)
    for g in range(4):
        sel[g, g * TC:(g + 1) * TC] = 1.0
    common["sel4"] = _bf16(sel)
    common["ident"] = _bf16(np.eye(H, dtype=f32))
    common["fcwt"] = _bf16(np.asarray(fc_w, f32).T / STEPS)
    common["fcb"] = _bf16(np.asarray(fc_b, f32)[None, :])
    common["gamma"] = np.ascontiguousarray(np.asarray(bn_gamma, f32)[:, None], f32)
    common["beta"] = np.ascontiguousarray(np.asarray(bn_beta, f32)[:, None], f32)

    in_maps = []
    for k in range(NCORES):
        xw = xp[:, 64 * k: 64 * k + 66, :]                     # [B, 66, C]
        taps = [xw[:, kk:kk + 64, :].transpose(2, 0, 1).reshape(C, B * TC)
                for kk in range(3)]                            # 3 x [14, B*64]
        arr = np.concatenate(taps, axis=0)                     # [42, B*64]
        hi = arr.astype(ml_dtypes.bfloat16)
        lo = (arr - hi.astype(f32)).astype(ml_dtypes.bfloat16)
        ones = np.ones((1, B * TC), ml_dtypes.bfloat16)
        m = dict(common)
        m["xt3"] = np.ascontiguousarray(np.concatenate(
            [ones, hi, lo], axis=0))                           # [85, B*64]
        in_maps.append(m)
    return in_maps


_CACHE = {}


def kernel(x, conv_w, conv_b, w_ih1, w_hh1, b_ih1, b_hh1, thr1,
           w_ih2, w_hh2, b_ih2, b_hh2, thr2, bn_gamma, bn_beta,
           fc_w, fc_b):
    thr1 = float(np.asarray(thr1)); thr2 = float(np.asarray(thr2))
    key = (thr1, thr2)
    if key not in _CACHE:
        _CACHE[key] = build_kernel(thr1, thr2)
    nc = _CACHE[key]
    in_maps = _host_inputs(x, conv_w, conv_b, w_ih1, w_hh1, b_ih1, b_hh1,
                           w_ih2, w_hh2, b_ih2, b_hh2, bn_gamma, bn_beta,
                           fc_w, fc_b)
    res = run_bass_kernel_spmd(nc, in_maps, core_ids=list(range(NCORES)),
                               trace=bool(int(os.environ.get("SLSTM_TRACE", "0"))))
    outT = np.concatenate([r["out"] for r in res.results], axis=1)  # [8, 512]
    if res.exec_time_ns is not None:
        kernel.last_exec_time_ns = res.exec_time_ns
    return np.ascontiguousarray(outT.T.astype(np.float32))


# revision 64
# speedup vs baseline: 1.0883x; 1.0883x over previous
"""Trainium2 Bass kernel for nn_Net_SLSTM_Conv (conv1d -> spiking LSTM -> BN ->
spiking LSTM -> mean -> fc), data-parallel over the T=512 axis on 8 cores.

Layout strategy (per core, T-chunk of 64 columns, processed as TWO
interleaved independent 32-column half-chains):
  - Everything feature-major: [features on partitions, t-columns on free dim].
  - The per-step recurrence latency is dominated by fixed per-instruction
    memory-access/semaphore latencies, so two staggered half-width chains
    overlap each other's dependency gaps on the engines (~8% wall win).
  - Conv1d folded into one K=85 matmul (bf16 hi/lo split of x + ones row for
    bias); the xt3 DMA is chunked and conv matmuls are interleaved into the
    scan so step 0 starts as soon as chunk 0 lands.
  - Gate preactivations accumulate in a per-step PSUM bank [128, 4*64]
    (gates ordered g,i,f,o; gate g pre-scaled by 2 so one sigmoid op serves
    all four gates: tanh(x) = 2*sigmoid(2x)-1).
  - mem = o*tanh(syn) - thr*spk_prev is NEVER materialized: the recurrent
    matmul is split into Whh@mp (mp = o*tanh(syn), on the critical chain)
    plus (-thr*Whh)@spk_prev (off-chain, spikes known one step earlier).
    This drops one DVE op from the per-step dependency cycle.
  - Layer-1 spike counts accumulate per-step via a 64-column reduce placed
    in the DVE's idle window (no 17us end-of-scan reduce); BN normalization
    folds into layer-2 input weights/bias entirely on device (transpose
    matmul against a host identity; no DRAM round-trip).
  - fc bias enters the fc PSUM accumulation as a K=1 matmul against the
    xt3 ones row; the output DMAs from a plain DVE copy of the PSUM bank.
  - mean-over-steps + fc fold into an accumulating K=128->M=8 matmul pair
    (fcw@mp and -thr*fcw@spk).
"""
import os
import numpy as np
import ml_dtypes

import concourse.bass as bass
import concourse.mybir as mybir
import concourse.tile as tile
from concourse.bass_utils import run_bass_kernel_spmd

BF = mybir.dt.bfloat16
F32 = mybir.dt.float32
AF = mybir.ActivationFunctionType
OP = mybir.AluOpType

NCORES = 8
B, T, C = 256, 512, 14
H = 128
CH = 32          # conv output channels
TC = T // NCORES  # 64 t-columns per core
STEPS = int(os.environ.get("SLSTM_STEPS", B))  # debug override
EPS = 1e-5
GBUFS = 2        # PSUM step-bank rotation depth
NCONV = (B * TC) // 512       # conv chunks of 512 columns (= 8 steps each)
NDMA = 8                      # xt3 DMA chunks


def _bf16(x):
    return np.asarray(x, np.float32).astype(ml_dtypes.bfloat16)


def _reorder_gates_cols(wt):
    # [*, 512] gate-major cols in torch order i,f,g,o -> (g,i,f,o), scale g by 2
    i, f, g, o = (wt[..., k * H:(k + 1) * H] for k in range(4))
    return np.concatenate([2.0 * g, i, f, o], axis=-1)


def build_kernel(thr1: float, thr2: float):
    assert thr1 == 1.0 and thr2 == 1.0, "kernel specialized for thr == 1.0"
    nc = bass.Bass()

    # ---- external I/O ----
    xt3_d = nc.dram_tensor("xt3", [85, B * TC], BF, kind="ExternalInput")
    wconv_d = nc.dram_tensor("wconv", [85, CH], BF, kind="ExternalInput")
    w1t_d = nc.dram_tensor("w1t", [32, 4 * H], BF, kind="ExternalInput")
    w1b_d = nc.dram_tensor("w1b", [1, 4 * H], BF, kind="ExternalInput")
    whh1t_d = nc.dram_tensor("whh1t", [H, 4 * H], BF, kind="ExternalInput")
    w2t32_d = nc.dram_tensor("w2t32", [H, 4 * H], F32, kind="ExternalInput")
    w2tbf_d = nc.dram_tensor("w2tbf", [H, 4 * H], BF, kind="ExternalInput")
    whh2t_d = nc.dram_tensor("whh2t", [H, 4 * H], BF, kind="ExternalInput")
    b2sum4_d = nc.dram_tensor("b2sum4", [H, 4], F32, kind="ExternalInput")
    sel4_d = nc.dram_tensor("sel4", [4, 4 * (TC // 2)], BF, kind="ExternalInput")
    ident_d = nc.dram_tensor("ident", [H, H], BF, kind="ExternalInput")
    fcwt_d = nc.dram_tensor("fcwt", [H, 8], BF, kind="ExternalInput")
    fcb_d = nc.dram_tensor("fcb", [1, 8], BF, kind="ExternalInput")
    gamma_d = nc.dram_tensor("gamma", [H, 1], F32, kind="ExternalInput")
    beta_d = nc.dram_tensor("beta", [H, 1], F32, kind="ExternalInput")
    out_d = nc.dram_tensor("out", [8, TC], F32, kind="ExternalOutput")
    DBG = bool(int(os.environ.get("SLSTM_DEBUG", "0")))
    if DBG:
        spk0_dd = nc.dram_tensor("spk0_d", [CH, B * TC], BF, kind="ExternalOutput")
        spk1_dd = nc.dram_tensor("spk1_d", [H, B * TC], BF, kind="ExternalOutput")
        cnt_dd = nc.dram_tensor("cnt_d", [H, 1], F32, kind="ExternalOutput")
        b2p_dd = nc.dram_tensor("b2p_d", [4, H], BF, kind="ExternalOutput")
        w2e_dd = nc.dram_tensor("w2e_d", [H, 4 * H], BF, kind="ExternalOutput")

    with tile.TileContext(nc) as tc:
        import contextlib
        ctx = contextlib.ExitStack()
        with ctx:
            const = ctx.enter_context(tc.tile_pool(name="const", bufs=1))
            big = ctx.enter_context(tc.tile_pool(name="big", bufs=1))
            spool = ctx.enter_context(tc.tile_pool(name="spool", bufs=3))
            vpool = ctx.enter_context(tc.tile_pool(name="vpool", bufs=3))
            stpool = ctx.enter_context(tc.tile_pool(name="stpool", bufs=3))
            gpool = ctx.enter_context(
                tc.tile_pool(name="gpool", bufs=GBUFS, space="PSUM"))
            cpool = ctx.enter_context(
                tc.tile_pool(name="cpool", bufs=2, space="PSUM"))
            fpool = ctx.enter_context(
                tc.tile_pool(name="fpool", bufs=1, space="PSUM"))
            dram = ctx.enter_context(
                tc.tile_pool(name="dram", bufs=1, space="DRAM"))

            # ---- load constants ----
            def load(pool, dt_, dram_t, shape):
                t_ = pool.tile(shape, dt_, name=dram_t.name + "_sb")
                nc.sync.dma_start(t_[:], dram_t[:])
                return t_

            # scan-critical loads first: conv weights + first xt3 chunk,
            # then layer-1 weights; everything else can trickle in behind.
            xt3_sb = big.tile([85, B * TC], BF, name="xt3_sb")
            # first chunk is tiny (just conv chunk 0) so step 0 starts ASAP
            XB = [0, 512, 2048, 4096, 6144, 8192, 10240, 12288, 14336, B * TC]
            def xt3_chunk(d):
                sl = slice(XB[d], XB[d + 1])
                nc.sync.dma_start(xt3_sb[:, sl], xt3_d[:, sl])
            xt3_chunk(0)
            wconv_sb = load(const, BF, wconv_d, [85, CH])
            w1t_sb = load(const, BF, w1t_d, [32, 4 * H])
            w1b_sb = load(const, BF, w1b_d, [1, 4 * H])
            xt3_chunk(1)
            whh1t_sb = load(const, BF, whh1t_d, [H, 4 * H])
            xt3_chunk(2)
            w2t32_sb = load(const, F32, w2t32_d, [H, 4 * H])
            w2tbf_sb = load(const, BF, w2tbf_d, [H, 4 * H])
            whh2t_sb = load(const, BF, whh2t_d, [H, 4 * H])
            b2sum4_sb = load(const, F32, b2sum4_d, [H, 4])
            xt3_chunk(3)
            sel4_sb = load(const, BF, sel4_d, [4, 4 * (TC // 2)])
            ident_sb = load(const, BF, ident_d, [H, H])
            fcwt_sb = load(const, BF, fcwt_d, [H, 8])
            fcb_sb = load(const, BF, fcb_d, [1, 8])
            gamma_sb = load(const, F32, gamma_d, [H, 1])
            beta_sb = load(const, F32, beta_d, [H, 1])
            for d in range(4, len(XB) - 1):
                xt3_chunk(d)

            spk0_sb = big.tile([CH, B * TC], BF, name="spk0")
            spk1_sb = big.tile([H, B * TC], BF, name="spk1")
            zeros_sb = const.tile([H, TC], BF, name="zeros")
            nc.vector.memset(zeros_sb[:], 0.0)
            cntc_sb = const.tile([H, 2 * STEPS], F32, name="cntc")

            # ---- conv chunk: matmul + spike threshold ----
            def conv_chunk(cchunk):
                cp = cpool.tile([CH, 512], F32, name="convp", tag="convp")
                sl = slice(cchunk * 512, (cchunk + 1) * 512)
                nc.tensor.matmul(cp[:, :], wconv_sb[:, :], xt3_sb[:, sl],
                                 start=True, stop=True)
                for hh in range(2):
                    hsl = slice(cchunk * 512 + hh * 256,
                                cchunk * 512 + (hh + 1) * 256)
                    nc.vector.tensor_scalar(spk0_sb[0:CH, hsl],
                                            cp[:, hh * 256:(hh + 1) * 256],
                                            1.0, 0.0, OP.subtract, OP.is_gt)

            conv_chunk(0)
            # derived constants, after the startup-critical conv work
            whh1s_sb = const.tile([H, 4 * H], BF, name="whh1s")
            nc.vector.tensor_scalar(whh1s_sb[:], whh1t_sb[:], -thr1, None, OP.mult)
            save = tc.cur_priority
            tc.cur_priority = save + 500000
            for c in range(1, min(4, NCONV)):
                conv_chunk(c)
            tc.cur_priority = save

            # ---- the recurrent step (per half-chain: HT=32 columns) ----
            # Two independent half-width chains (cols 0:32 / 32:64 of each
            # step slice) interleave on the engines: each chain's per-step
            # latency is lower (less compute per fixed-cost visit) and the
            # two chains hide each other's memory-ack/semaphore gaps.
            HT = TC // 2
            def lstm_step(b, hf, layer, syn_prev, mp_prev, spk_prev,
                          spk_prev2, thr):
                c0 = b * TC + hf * HT          # global column base
                gb = gpool.tile([H, 4 * HT], F32, name="gbank",
                                tag=f"gbank{hf}")
                ones = xt3_sb[0:1, c0:c0 + HT]
                started = False
                if layer == 2:
                    nc.tensor.matmul(gb[:, :], b2p_sb[:, :], sel4_sb[:, :],
                                     start=True, stop=False,
                                     skip_group_check=True)
                    started = True
                    rhs_in = spk1_sb[:, c0:c0 + HT]
                else:
                    rhs_in = spk0_sb[:, c0:c0 + HT]
                for g in range(4):
                    lhs = (w2eff_sb[:, g * H:(g + 1) * H] if layer == 2
                           else w1t_sb[:, g * H:(g + 1) * H])
                    nc.tensor.matmul(gb[:, g * HT:(g + 1) * HT],
                                     lhs, rhs_in,
                                     start=not started and g == 0,
                                     stop=(layer == 2 and b == 0 and g == 3),
                                     skip_group_check=True)
                if layer == 1:
                    for g in range(4):
                        nc.tensor.matmul(gb[:, g * HT:(g + 1) * HT],
                                         w1b_sb[:, g * H:(g + 1) * H],
                                         ones, start=False,
                                         stop=(b == 0 and g == 3),
                                         skip_group_check=True)
                whh = whh1t_sb if layer == 1 else whh2t_sb
                whs = whh1s_sb if layer == 1 else whh2s_sb
                if b >= 2:
                    for g in range(4):
                        nc.tensor.matmul(gb[:, g * HT:(g + 1) * HT],
                                         whs[:, g * H:(g + 1) * H],
                                         spk_prev2[:, :],
                                         start=False, stop=False,
                                         skip_group_check=True)
                if b >= 1:
                    for g in range(4):
                        nc.tensor.matmul(gb[:, g * HT:(g + 1) * HT],
                                         whh[:, g * H:(g + 1) * H],
                                         mp_prev[:, :],
                                         start=False, stop=(g == 3),
                                         skip_group_check=True)
                # gate order in bank: g' | i | f | o
                S = spool.tile([H, 4 * HT], BF, name="S", tag=f"S{hf}")
                nc.scalar.activation(S[:, 0:3 * HT], gb[:, 0:3 * HT],
                                     AF.Sigmoid)
                nc.scalar.activation(S[:, 3 * HT:], gb[:, 3 * HT:],
                                     AF.Sigmoid)
                # critical DVE block; state h == syn/2 (see tanh scale)
                syn = stpool.tile([H, HT], BF, name="syn", tag=f"syn{hf}")
                u = vpool.tile([H, HT], BF, name="u", tag=f"u{hf}")
                if b >= 1:
                    fs = vpool.tile([H, HT], BF, name="fs", tag=f"fs{hf}")
                    nc.vector.tensor_tensor(fs[:], S[:, 2 * HT:3 * HT],
                                            syn_prev[:], op=OP.mult)
                    nc.vector.scalar_tensor_tensor(
                        u[:], S[:, 0:HT], 0.5, S[:, HT:2 * HT],
                        op0=OP.subtract, op1=OP.mult)
                    nc.vector.tensor_tensor(syn[:], u[:], fs[:], op=OP.add)
                else:
                    nc.vector.scalar_tensor_tensor(
                        u[:], S[:, 0:HT], 0.5, S[:, HT:2 * HT],
                        op0=OP.subtract, op1=OP.mult)
                    nc.vector.tensor_tensor(syn[:], u[:],
                                            zeros_sb[:, 0:HT], op=OP.add)
                ts = vpool.tile([H, HT], BF, name="ts", tag=f"ts{hf}")
                nc.scalar.activation(ts[:], syn[:], AF.Tanh, scale=2.0)
                mp = stpool.tile([H, HT], BF, name="mp", tag=f"mp{hf}")
                nc.vector.tensor_tensor(mp[:], S[:, 3 * HT:4 * HT], ts[:],
                                        op=OP.mult)
                # spike off the critical chain
                if layer == 1:
                    spk = spk1_sb[:, c0:c0 + HT]
                else:
                    spk = stpool.tile([H, HT], BF, name="spk2",
                                      tag=f"spk2{hf}")[:]
                if b >= 1:
                    nc.vector.scalar_tensor_tensor(
                        spk, mp[:], thr, spk_prev, op0=OP.subtract,
                        op1=OP.is_gt)
                else:
                    nc.vector.tensor_scalar(spk, mp[:], thr, 0.0,
                                            OP.subtract, OP.is_gt)
                if layer == 1:
                    nc.vector.tensor_reduce(cntc_sb[:, 2 * b + hf:
                                                    2 * b + hf + 1], spk,
                                            axis=mybir.AxisListType.X,
                                            op=OP.add)
                return syn, mp, spk

            # ---- phase A: layer-1 scan (two interleaved half-chains) ----
            zh = zeros_sb[:, 0:TC // 2]
            st = [[zh, zh, zh, zh], [zh, zh, zh, zh]]  # per-half (syn,mp,spk,spk2)
            for b in range(STEPS):
                with tc.high_priority():
                    for hf in (0, 1):
                        syn_p, mp_p, spk_p, spk_p2 = st[hf]
                        syn_n, mp_n, spk_n = lstm_step(
                            b, hf, 1, syn_p, mp_p, spk_p, spk_p2, thr1)
                        st[hf] = [syn_n, mp_n, spk_n, spk_p]
                c = b // 8 + 4
                if b % 8 == 0 and c < NCONV:
                    # deprioritize conv threshold work vs the chain
                    save = tc.cur_priority
                    tc.cur_priority = save + 500000
                    conv_chunk(c)
                    tc.cur_priority = save

            # ---- BN stats: count -> AllReduce -> fold into layer-2 weights ----
            cnt = const.tile([H, 1], F32, name="cnt")
            nc.vector.tensor_reduce(cnt[:], cntc_sb[:, 0:2 * STEPS],
                                    axis=mybir.AxisListType.X, op=OP.add)
            cc_in = dram.tile([H, 1], F32, name="cc_in")
            cc_out = dram.tile([H, 1], F32, name="cc_out", addr_space="Shared")
            nc.sync.dma_start(cc_in[:], cnt[:])
            nc.gpsimd.collective_compute(
                "AllReduce", OP.add,
                replica_groups=[list(range(NCORES))],
                ins=[cc_in[:]], outs=[cc_out[:]])
            cntg = const.tile([H, 1], F32, name="cntg")
            nc.sync.dma_start(cntg[:], cc_out[:])

            p_t = const.tile([H, 1], F32, name="p_t")
            nc.vector.tensor_scalar(p_t[:], cntg[:], 1.0 / (B * T), None, OP.mult)
            nv_t = const.tile([H, 1], F32, name="nv_t")
            nc.vector.scalar_tensor_tensor(nv_t[:], p_t[:], 1.0, p_t[:],
                                           op0=OP.subtract, op1=OP.mult)
            var_t = const.tile([H, 1], F32, name="var_t")
            nc.vector.tensor_scalar(var_t[:], nv_t[:], -1.0, EPS,
                                    OP.mult, OP.add)
            sq_t = const.tile([H, 1], F32, name="sq_t")
            nc.scalar.activation(sq_t[:], var_t[:], AF.Sqrt, bias=0.0)
            rs_t = const.tile([H, 1], F32, name="rs_t")
            nc.vector.reciprocal(rs_t[:], sq_t[:])
            a_t = const.tile([H, 1], F32, name="a_t")
            nc.vector.tensor_tensor(a_t[:], gamma_sb[:], rs_t[:], op=OP.mult)
            pa_t = const.tile([H, 1], F32, name="pa_t")
            nc.vector.tensor_tensor(pa_t[:], p_t[:], a_t[:], op=OP.mult)
            c_t = const.tile([H, 1], F32, name="c_t")
            nc.vector.scalar_tensor_tensor(c_t[:], pa_t[:], -1.0, beta_sb[:],
                                           op0=OP.mult, op1=OP.add)
            cbf_t = const.tile([H, 1], BF, name="cbf_t")
            nc.vector.tensor_copy(cbf_t[:], c_t[:])

            w2eff_sb = const.tile([H, 4 * H], BF, name="w2eff")
            nc.vector.tensor_scalar(w2eff_sb[:], w2t32_sb[:], a_t[:], None, OP.mult)
            whh2s_sb = const.tile([H, 4 * H], BF, name="whh2s")
            nc.vector.tensor_scalar(whh2s_sb[:], whh2t_sb[:], -thr2, None, OP.mult)
            fcwts_sb = const.tile([H, 8], BF, name="fcwts")
            nc.vector.tensor_scalar(fcwts_sb[:], fcwt_sb[:], -thr2, None, OP.mult)

            # layer-2 bias = W2 @ c + b2sum, built fully on device:
            # bias2[h, g] via 4 single-column matmuls, + b2sum4; then
            # transpose to [4, H] with a matmul against the identity.
            bias2p = cpool.tile([H, 4], F32, name="bias2p", tag="convp")
            for g in range(4):
                nc.tensor.matmul(bias2p[:, g:g + 1],
                                 w2tbf_sb[:, g * H:(g + 1) * H], cbf_t[:, :],
                                 start=True, stop=True)
            bias2bf = const.tile([H, 4], BF, name="bias2bf")
            nc.vector.tensor_tensor(bias2bf[:], bias2p[:, :], b2sum4_sb[:],
                                    op=OP.add)
            b2pp = cpool.tile([4, H], F32, name="b2pp", tag="convp")
            nc.tensor.matmul(b2pp[:, :], bias2bf[:, :], ident_sb[:, :],
                             start=True, stop=True)
            b2p_sb = const.tile([4, H], BF, name="b2p")
            nc.vector.tensor_copy(b2p_sb[:], b2pp[:, :])

            # ---- phase B: layer-2 scan, fused mean+fc accumulation ----
            HT = TC // 2
            fcps = []
            for hf in (0, 1):
                t = fpool.tile([8, HT], F32, name=f"fcp{hf}", tag=f"fcp{hf}")
                nc.tensor.matmul(t[:, :], fcb_sb[:, :],
                                 xt3_sb[0:1, hf * HT:(hf + 1) * HT],
                                 start=True, stop=False)
                fcps.append(t)

            def fc_mms(b, hf, mp_b, spk_bm1):
                nc.tensor.matmul(fcps[hf][:, :], fcwt_sb[:, :], mp_b[:, :],
                                 start=False, stop=False)
                if b >= 1:
                    nc.tensor.matmul(fcps[hf][:, :], fcwts_sb[:, :], spk_bm1,
                                     start=False, stop=(b == STEPS - 1))

            zh = zeros_sb[:, 0:HT]
            st = [[zh, zh, zh, zh], [zh, zh, zh, zh]]
            pend = [None, None]
            for b in range(STEPS):
                for hf in (0, 1):
                    syn_p, mp_p, spk_p, spk_p2 = st[hf]
                    pend[hf] = (b - 1, hf, mp_p, spk_p2)
                    syn_n, mp_n, spk_n = lstm_step(
                        b, hf, 2, syn_p, mp_p, spk_p, spk_p2, thr2)
                    st[hf] = [syn_n, mp_n, spk_n, spk_p]
                    if b >= 1:
                        fc_mms(*pend[hf])
            for hf in (0, 1):
                fc_mms(STEPS - 1, hf, st[hf][1], st[hf][3])

            out_sb = const.tile([8, TC], F32, name="out_sb")
            nc.vector.tensor_copy(out_sb[:, 0:HT], fcps[0][:, :])
            nc.vector.tensor_copy(out_sb[:, HT:TC], fcps[1][:, :])
            nc.sync.dma_start(out_d[:], out_sb[:])

            if DBG:
                nc.sync.dma_start(spk0_dd[:], spk0_sb[:])
                nc.sync.dma_start(spk1_dd[:], spk1_sb[:])
                nc.sync.dma_start(cnt_dd[:], cnt[:])
                nc.sync.dma_start(b2p_dd[:], b2p_sb[:])
                nc.sync.dma_start(w2e_dd[:], w2eff_sb[:])

    _split_mm_waits(nc)
    return nc


def _split_mm_waits(nc):
    """Most ISA structs carry a single sync-wait slot, so extra Tile-assigned
    waits must move onto preceding same-engine NoOps. Choose the KEPT wait to
    be the one whose producing update appears LATEST in program order (the
    likely critical dependency); early-satisfied waits go to the NoOps, which
    then drain instantly instead of serializing the critical path."""
    for fn in nc.m.functions:
        for blk in fn.blocks:
            # pass 1: per-semaphore cumulative update positions
            sem_updates = {}  # (name,id) -> list of (cum_value, pos)
            for pos, inst in enumerate(blk.instructions):
                si = getattr(inst, "sync_info", None)
                if si is None:
                    continue
                for upd in (si.on_update or []):
                    key = (upd.ant_name, upd.id)
                    lst = sem_updates.setdefault(key, [])
                    prev = lst[-1][0] if lst else 0
                    val = getattr(upd, "value", 1) or 1
                    lst.append((prev + val, pos))

            def producer_pos(w):
                key = (w.ant_name, w.id)
                lst = sem_updates.get(key)
                tgt = getattr(w, "wait_value", None)
                if not lst or tgt is None:
                    return -1
                lo, hi = 0, len(lst)
                while lo < hi:
                    mid = (lo + hi) // 2
                    if lst[mid][0] >= tgt:
                        hi = mid
                    else:
                        lo = mid + 1
                return lst[lo][1] if lo < len(lst) else 10 ** 9

            out = []
            for inst in blk.instructions:
                si = getattr(inst, "sync_info", None)
                if (not isinstance(inst, (mybir.InstEventSemaphore,
                                          mybir.InstAllEngineBarrier,
                                          mybir.InstNoOp))
                        and si is not None and si.on_wait
                        and len(si.on_wait) > 1):
                    waits = sorted(si.on_wait, key=producer_pos)
                    for j, w in enumerate(waits[:-1]):
                        nop = mybir.InstNoOp(name=f"{inst.name}-wsplit{j}",
                                             ins=[], outs=[])
                        nop.engine = inst.engine
                        nop.sync_info = mybir.SyncInfo(on_wait=[w],
                                                       on_update=[])
                        out.append(nop)
                    si.on_wait = [waits[-1]]
                out.append(inst)
            blk.instructions[:] = out


def _host_inputs(x, conv_w, conv_b, w_ih1, w_hh1, b_ih1, b_hh1,
                 w_ih2, w_hh2, b_ih2, b_hh2, bn_gamma, bn_beta, fc_w, fc_b):
    """Build the per-core input dicts (numpy, host-side)."""
    f32 = np.float32
    # im2col with hi/lo bf16 split per core
    xp = np.pad(np.asarray(x, f32), ((0, 0), (1, 1), (0, 0)))  # [B, T+2, C]
    common = {}
    w3t = np.concatenate([conv_w[:, :, k].T for k in range(3)], axis=0)  # [42,32]
    common["wconv"] = _bf16(np.concatenate(
        [np.asarray(conv_b, f32)[None, :], w3t, w3t], axis=0))
    w1t = _reorder_gates_cols(np.asarray(w_ih1, f32).T)        # [32, 512]
    b1 = _reorder_gates_cols((np.asarray(b_ih1) + np.asarray(b_hh1))[None, :])
    common["w1t"] = _bf16(w1t)                                 # [32, 512]
    common["w1b"] = _bf16(b1)                                  # [1, 512]
    common["whh1t"] = _bf16(_reorder_gates_cols(np.asarray(w_hh1, f32).T))
    w2t = _reorder_gates_cols(np.asarray(w_ih2, f32).T)        # [128, 512]
    common["w2t32"] = np.ascontiguousarray(w2t, f32)
    common["w2tbf"] = _bf16(w2t)
    common["whh2t"] = _bf16(_reorder_gates_cols(np.asarray(w_hh2, f32).T))
    b2r = _reorder_gates_cols((np.asarray(b_ih2) + np.asarray(b_hh2))[None, :])
    common["b2sum4"] = np.ascontiguousarray(b2r.reshape(4, H).T, f32)  # [H, 4]
    HTC = TC // 2
    sel = np.zeros((4, 4 * HTC), f32)
    for g in range(4):
        sel[g, g * HTC:(g + 1) * HTC] = 1.0
    common["sel4"] = _bf16(sel)
    common["ident"] = _bf16(np.eye(H, dtype=f32))
    common["fcwt"] = _bf16(np.asarray(fc_w, f32).T / STEPS)
    common["fcb"] = _bf16(np.asarray(fc_b, f32)[None, :])
    common["gamma"] = np.ascontiguousarray(np.asarray(bn_gamma, f32)[:, None], f32)
    common["beta"] = np.ascontiguousarray(np.asarray(bn_beta, f32)[:, None], f32)

    in_maps = []
    for k in range(NCORES):
        xw = xp[:, 64 * k: 64 * k + 66, :]                     # [B, 66, C]
        taps = [xw[:, kk:kk + 64, :].transpose(2, 0, 1).reshape(C, B * TC)
                for kk in range(3)]                            # 3 x [14, B*64]
        arr = np.concatenate(taps, axis=0)                     # [42, B*64]
        hi = arr.astype(ml_dtypes.bfloat16)
        lo = (arr - hi.astype(f32)).astype(ml_dtypes.bfloat16)
        ones = np.ones((1, B * TC), ml_dtypes.bfloat16)
        m = dict(common)
        m["xt3"] = np.ascontiguousarray(np.concatenate(
            [ones, hi, lo], axis=0))                           # [85, B*64]
        in_maps.append(m)
    return in_maps


_CACHE = {}


def kernel(x, conv_w, conv_b, w_ih1, w_hh1, b_ih1, b_hh1, thr1,
           w_ih2, w_hh2, b_ih2, b_hh2, thr2, bn_gamma, bn_beta,
           fc_w, fc_b):
    thr1 = float(np.asarray(thr1)); thr2 = float(np.asarray(thr2))
    key = (thr1, thr2)
    if key not in _CACHE:
        _CACHE[key] = build_kernel(thr1, thr2)
    nc = _CACHE[key]
    in_maps = _host_inputs(x, conv_w, conv_b, w_ih1, w_hh1, b_ih1, b_hh1,
                           w_ih2, w_hh2, b_ih2, b_hh2, bn_gamma, bn_beta,
                           fc_w, fc_b)
    res = run_bass_kernel_spmd(nc, in_maps, core_ids=list(range(NCORES)),
                               trace=bool(int(os.environ.get("SLSTM_TRACE", "0"))))
    outT = np.concatenate([r["out"] for r in res.results], axis=1)  # [8, 512]
    if res.exec_time_ns is not None:
        kernel.last_exec_time_ns = res.exec_time_ns
    return np.ascontiguousarray(outT.T.astype(np.float32))
